# revision 29
# baseline (speedup 1.0000x reference)
"""CompressiveMemory (Infini-attention style) Bass kernel for 8x TRN2 NeuronCores.

Problem (hardcoded): B=4, T=4096, D=2048, H=16, DK=DV=128, SEG=512, NSEG=8.

The reference's _split_heads is a RAW reshape (B, SEG, H*dk) -> (B, H, SEG, dk):
head h's (SEG, dk) matrix is segment time rows [h*32, (h+1)*32) of the full
(SEG, 2048) projection, reinterpreted as 512 rows of 128. Attention is
permutation-equivariant over sequence positions, so we use the hardware-
friendly ordering s' = r*32 + tau (r = col-chunk 0..15, tau = time row 0..31);
q/k/v/att all use the same ordering and the result is identical.

Sharding: 8 cores = 4 batches x 2 head-halves. Core c handles batch c//2 and
heads (c%2)*8..+8, i.e. time rows [hh*256, (hh+1)*256) of every segment.
Output rows are disjoint across cores; the host just interleaves them.

v2 design notes (vs the first working version):
 - x is transposed on the host; phase 1 runs with 512-wide moving operands
   and no PE transposes (LDWEIGHTS hidden under matmuls).
 - v scratch and the whole att/Wo path are bf16 (PE same speed, half DMA);
   q/k scratch stays f32 for the softmax/elu path.
 - phase 2: denominators are computed PE-replicated (ones128 stationary, no
   extra PE cost since the moving dim is unchanged) and inverted with
   reciprocal_approx_fast on [128,512] tiles -- no single-partition
   reciprocals. (1-bg) and bg/(1+cum) column factors fold into the ScalarE
   PSUM->SBUF copies. The delta-rule matmul runs 256-wide over a
   [v | corr*rsk] concatenated rhs to dodge the fp32r narrow-moving penalty.
 - phase 3 (out = att @ Wo, bf16) is fused into the per-segment loop; att
   never round-trips to DRAM.

Math notes vs the reference:
 - z always has identical rows: z[., c] = 1 + cum[c]; only the cum vector is
   tracked. sq@z = outer(rowsum(sq), 1+cum), sk@z likewise.
 - softmax without max subtraction (scores*scale ~ N(0,1), exp safe in fp32).
 - elu(x)+1 = relu(x) + exp(-relu(-x)).
 - delta update: mem += sq^T@v - (sq^T@(corr*rsk_col))*r1_row, corr = sk@mem.
"""

import os
from contextlib import ExitStack

import ml_dtypes
import numpy as np

import concourse.bass as bass
import concourse.mybir as mybir
import concourse.tile as tile
from concourse import bacc
from concourse.bass_utils import run_bass_kernel_spmd

F32 = mybir.dt.float32
F32R = mybir.dt.float32r
BF16 = mybir.dt.bfloat16
AF = mybir.ActivationFunctionType
ALU = mybir.AluOpType

# Global problem dims
T = 4096
D = 2048
SEG = 512
NSEG = T // SEG
NH = 8            # local heads per core
DK = 128
DV = 128
TC = NSEG * 256   # 2048 core-local time rows (256 per segment)
NKK = D // 128    # 16 contraction chunks
SCALE = 1.0 / np.sqrt(128.0)


def build_core_program():
    phases = os.environ.get("BASS_PHASES", "12")
    nc = bacc.Bacc("TRN2", target_bir_lowering=False)

    xt = nc.dram_tensor("xt", [D, TC], BF16, kind="ExternalInput")
    wq = nc.dram_tensor("wq", [D, D], BF16, kind="ExternalInput")
    wk = nc.dram_tensor("wk", [D, D], BF16, kind="ExternalInput")
    wv = nc.dram_tensor("wv", [D, D], BF16, kind="ExternalInput")
    wo = nc.dram_tensor("wo", [D, D], BF16, kind="ExternalInput")
    bg_cols = nc.dram_tensor("bg_cols", [DV, NH], F32, kind="ExternalInput")
    omb_cols = nc.dram_tensor("omb_cols", [DV, NH], F32, kind="ExternalInput")
    ident_in = nc.dram_tensor("ident", [128, 128], F32, kind="ExternalInput")
    ones_in = nc.dram_tensor("ones_in", [128, 128], F32, kind="ExternalInput")
    ones_bf_in = nc.dram_tensor("ones_bf", [128, 128], BF16, kind="ExternalInput")
    ident_bf_in = nc.dram_tensor("ident_bf", [128, 128], BF16, kind="ExternalInput")
    zeros_in = nc.dram_tensor("zeros_in", [128, NH * DV + NH], F32, kind="ExternalInput")
    out = nc.dram_tensor("out", [TC, D], F32, kind="ExternalOutput")

    with ExitStack() as ctx:
        ctx.enter_context(
            nc.allow_low_precision(reason="float32r/bf16 tiles feed the PE")
        )
        tc = ctx.enter_context(tile.TileContext(nc))

        # ---- DRAM scratch ----
        dram = ctx.enter_context(tc.tile_pool(name="dram", bufs=1, space="DRAM"))
        qT_scr = dram.tile([NKK, 128, TC], BF16)  # P_q^T as (jc, dd, t)
        kT_scr = dram.tile([NKK, 128, TC], BF16)
        v_scr = dram.tile([TC, D], BF16)          # P_v natural, bf16

        # ---- constants ----
        const = ctx.enter_context(tc.tile_pool(name="const", bufs=1))
        ident = const.tile([128, 128], F32R)
        nc.sync.dma_start(out=ident, in_=ident_in[:, :].bitcast(F32R))
        ones128 = const.tile([128, 128], F32R)
        nc.sync.dma_start(out=ones128, in_=ones_in[:, :].bitcast(F32R))
        ones128_bf = const.tile([128, 128], BF16)
        nc.sync.dma_start(out=ones128_bf, in_=ones_bf_in[:, :])
        ones_row_bf = const.tile([1, 128], BF16)
        nc.sync.dma_start(out=ones_row_bf, in_=ones_bf_in[0:1, :])
        ident_bf = const.tile([128, 128], BF16)
        nc.sync.dma_start(out=ident_bf, in_=ident_bf_in[:, :])

        # =========================== PHASE 1 ===========================
        # Three passes; one full (D, D) projection weight resident per pass.
        # q/k emit transposed f32 scratch; v emits natural bf16 scratch.
        with ExitStack() as p1w:
          wpool = (
              p1w.enter_context(tc.tile_pool(name="w_p1", bufs=2))
              if "1" in phases
              else None
          )
          for which, w_dram in ((("q", wq), ("k", wk), ("v", wv)) if "1" in phases else ()):
            with ExitStack() as p1:
                sb = p1.enter_context(tc.tile_pool(name=f"sb_{which}", bufs=2))
                st_pool = p1.enter_context(tc.tile_pool(name=f"st_{which}", bufs=4))
                ps = p1.enter_context(tc.tile_pool(name=f"ps_{which}", bufs=5, space="PSUM"))

                wt = wpool.tile([128, NKK, D], BF16, tag="wt")
                w_view = w_dram.rearrange("(kk p) j -> p kk j", p=128)
                if which == "q":
                    # chunked: the first jc group starts after chunk 0 lands
                    for kk in range(NKK):
                        nc.gpsimd.dma_start(out=wt[:, kk, :], in_=w_view[:, kk, :])
                else:
                    nc.gpsimd.dma_start(out=wt, in_=w_view)

                for g in range(4):  # 512 t-cols per group
                    c0 = g * 512
                    xg = sb.tile([128, NKK, 512], BF16, tag="xg")
                    nc.sync.dma_start(
                        out=xg,
                        in_=xt[:, c0 : c0 + 512].rearrange("(kk p) t -> p kk t", p=128),
                    )
                    if which in ("q", "k"):
                        scr = qT_scr if which == "q" else kT_scr
                        for jc in range(NKK):
                            pq = ps.tile([128, 512], F32, tag="proj")
                            for kk in range(NKK):
                                nc.tensor.matmul(
                                    pq,
                                    wt[:, kk, jc * 128 : (jc + 1) * 128],
                                    xg[:, kk, :],
                                    start=(kk == 0),
                                    stop=(kk == NKK - 1),
                                )
                            qs = st_pool.tile([128, 512], BF16, tag="stage")
                            if jc % 2 == 0:
                                nc.scalar.copy(qs, pq)
                            else:
                                nc.vector.tensor_copy(qs, pq)
                            nc.sync.dma_start(out=scr[jc, :, c0 : c0 + 512], in_=qs)
                    else:
                        for tch in range(4):  # t-chunks of 128 within group
                            r0 = c0 + tch * 128
                            for mc in range(4):
                                pv = ps.tile([128, 512], F32, tag="proj")
                                for kk in range(NKK):
                                    nc.tensor.matmul(
                                        pv,
                                        xg[:, kk, tch * 128 : (tch + 1) * 128],
                                        wt[:, kk, mc * 512 : (mc + 1) * 512],
                                        start=(kk == 0),
                                        stop=(kk == NKK - 1),
                                    )
                                vs = st_pool.tile([128, 512], BF16, tag="stage")
                                if (tch + mc) % 2 == 0:
                                    nc.scalar.copy(vs, pv)
                                else:
                                    nc.vector.tensor_copy(vs, pv)
                                nc.sync.dma_start(
                                    out=v_scr[r0 : r0 + 128, mc * 512 : (mc + 1) * 512],
                                    in_=vs,
                                )

        # ================= PHASE 2+3 (fused, per segment) =================
        with ExitStack() as p2:
          if "2" in phases:
            res = p2.enter_context(tc.tile_pool(name="res", bufs=1))
            wpool3 = p2.enter_context(tc.tile_pool(name="w_o", bufs=1))
            qk = p2.enter_context(tc.tile_pool(name="qk", bufs=3))
            vcp = p2.enter_context(tc.tile_pool(name="vcp", bufs=2))
            etp = p2.enter_context(tc.tile_pool(name="etp", bufs=3))
            work = p2.enter_context(tc.tile_pool(name="work", bufs=2))
            segw = p2.enter_context(tc.tile_pool(name="segw", bufs=2))
            afp = p2.enter_context(tc.tile_pool(name="afp", bufs=2))
            outst = p2.enter_context(tc.tile_pool(name="outst", bufs=2))
            psA = p2.enter_context(tc.tile_pool(name="psA", bufs=2, space="PSUM"))
            psDen = p2.enter_context(tc.tile_pool(name="psDen", bufs=2, space="PSUM"))
            psDM = p2.enter_context(tc.tile_pool(name="psDM", bufs=2, space="PSUM"))
            psSm = p2.enter_context(tc.tile_pool(name="psSm", bufs=2, space="PSUM"))

            # residents
            wo_res = wpool3.tile([128, NKK, D], BF16, tag="wo")
            nc.sync.dma_start(out=wo_res, in_=wo.rearrange("(r p) m -> p r m", p=128))
            bgc = res.tile([128, NH], F32)
            nc.sync.dma_start(out=bgc, in_=bg_cols[:, :])
            ombc = res.tile([128, NH], F32)
            nc.sync.dma_start(out=ombc, in_=omb_cols[:, :])
            mem = res.tile([128, NH, DV], F32R)  # (dk, h, dv)
            nc.sync.dma_start(
                out=mem,
                in_=zeros_in[:, 0 : NH * DV].rearrange("p (h v) -> p h v", h=NH).bitcast(F32R),
            )
            cum = res.tile([128, NH], F32R)      # (dk, h)
            nc.sync.dma_start(out=cum, in_=zeros_in[:, NH * DV :].bitcast(F32R))

            for seg in range(NSEG):
                r0 = seg * 256
                # ---- per-segment loads (two head-halves of 4 heads) ----
                vcb = []
                for hf in range(2):
                    rows = slice(r0 + hf * 128, r0 + hf * 128 + 128)
                    # v | corrS combo tile; left 128 of each 256 slab = v
                    vc4 = vcp.tile([128, 4, 4, 256], BF16, tag="vc")
                    v_base = v_scr[rows, :].rearrange(
                        "(hh t) (c rr d) -> rr hh t c d", hh=4, c=4, rr=4, d=128
                    )
                    for rr in range(4):
                        for hh4 in range(4):
                            nc.gpsimd.dma_start(
                                out=vc4[rr * 32 : (rr + 1) * 32, hh4, :, 0:128],
                                in_=v_base[rr, hh4],
                            )
                    vcb.append(vc4)

                # ---- cum-derived factors for all heads (OLD cum) ----
                cumT_ps = psSm.tile([NH, 128], F32R, tag="sm")
                nc.tensor.transpose(cumT_ps, cum[:, :], ident)
                z1row8 = segw.tile([NH, 128], F32, tag="z1row8")
                nc.scalar.add(z1row8, cumT_ps, 1.0)
                r1row8_bf = segw.tile([NH, 128], BF16, tag="r1row8b")
                r1tmp = segw.tile([NH, 128], F32, tag="r1tmp")
                nc.vector.reciprocal_approx_fast(out=r1tmp, in_=z1row8)
                nc.vector.tensor_copy(r1row8_bf, r1tmp)
                # flatten to one partition so per-head slices sit at base 0
                r1flat = segw.tile([1, NH, 128], BF16, tag="r1flat")
                nc.sync.dma_start(out=r1flat, in_=r1row8_bf[:, :])
                z1col = segw.tile([128, NH], F32, tag="z1col")
                nc.scalar.add(z1col, cum[:, :], 1.0)
                r1col = segw.tile([128, NH], F32, tag="r1col")
                nc.vector.reciprocal_approx_fast(out=r1col, in_=z1col)
                bgr1 = segw.tile([128, NH], F32, tag="bgr1")
                nc.vector.tensor_mul(bgr1, bgc, r1col)

                # assembled att_flat^T for this segment (bf16)
                af = afp.tile([128, NKK, 256], BF16, tag="af")

                rep_all = []
                for hf in range(2):
                    rep_ps4 = psDen.tile([128, SEG], F32, tag="ps")
                    nc.tensor.matmul(
                        rep_ps4,
                        ones_row_bf,
                        r1flat[:, hf * 4 : hf * 4 + 4, :].rearrange("o h d -> o (h d)"),
                        start=True,
                        stop=True,
                    )
                    rep4 = segw.tile([128, 4, 128], F32, tag=f"rep4_{hf}")
                    nc.scalar.copy(rep4, rep_ps4)
                    rep_all.append(rep4)

                for h in range(NH):
                    hf, hh = h // 4, h % 4
                    tr0 = r0 + hf * 128 + hh * 32
                    vc = vcb[hf][:, hh]          # [128, 4, 256] bf16

                    # per-head q/k loads (s' = r*32 + tau ordering, contiguous)
                    qT3 = qk.tile([128, NKK, 32], BF16, tag="qT")
                    nc.gpsimd.dma_start(
                        out=qT3,
                        in_=qT_scr[:, :, tr0 : tr0 + 32].rearrange("r d t -> d r t"),
                    )
                    kT3 = qk.tile([128, NKK, 32], BF16, tag="kT")
                    nc.gpsimd.dma_start(
                        out=kT3,
                        in_=kT_scr[:, :, tr0 : tr0 + 32].rearrange("r d t -> d r t"),
                    )
                    qT = qT3.rearrange("p r t -> p (r t)")

                    # ---- sq/sk = elu+1 ----
                    sqT = work.tile([128, SEG], BF16, tag="sqT")
                    skT = work.tile([128, SEG], BF16, tag="skT")
                    for which_t, dst in (("q", sqT), ("k", skT)):
                        src = qT if which_t == "q" else kT3.rearrange("p r t -> p (r t)")
                        nrelu = work.tile([128, SEG], F32, tag="nrelu")
                        nc.scalar.activation(out=nrelu, in_=src, func=AF.Relu, scale=-1.0)
                        texp = work.tile([128, SEG], F32, tag="texp")
                        nc.scalar.activation(out=texp, in_=nrelu, func=AF.Exp, scale=-1.0)
                        nc.vector.scalar_tensor_tensor(
                            out=dst, in0=src, scalar=0.0, in1=texp,
                            op0=ALU.max, op1=ALU.add,
                        )

                    # cum += colsum(sk); the z1/bgr1 reads above used OLD cum
                    csk = work.tile([128, 1], F32, tag="csk")
                    nc.vector.reduce_sum(csk, skT, axis=mybir.AxisListType.X)
                    nc.vector.tensor_add(cum[:, h : h + 1], cum[:, h : h + 1], csk)

                    # mem in bf16 for the corr matmul
                    mem_bf = work.tile([128, DV], BF16, tag="mem_bf")
                    nc.scalar.copy(mem_bf, mem[:, h, :])

                    # ---- scores^T -> e^T (bf16) ----
                    eT = etp.tile([128, 4, SEG], BF16, tag="eT")
                    for c in range(4):
                        pscT = psA.tile([128, SEG], F32, tag="ps")
                        nc.tensor.matmul(
                            pscT,
                            kT3[:, c * 4 : c * 4 + 4, :].rearrange("p r t -> p (r t)"),
                            qT,
                            start=True,
                            stop=True,
                        )
                        nc.scalar.activation(out=eT[:, c, :], in_=pscT, func=AF.Exp, scale=SCALE)

                    # ---- replicated denominators ----
                    den_ps = psDen.tile([128, SEG], F32, tag="ps")
                    for c in range(4):
                        nc.tensor.matmul(
                            den_ps, ones128_bf, eT[:, c, :], start=(c == 0), stop=(c == 3)
                        )
                    r_dot = work.tile([128, SEG], F32, tag="r_dot")
                    nc.vector.reciprocal_approx_fast(out=r_dot, in_=den_ps)

                    rsq_ps = psDen.tile([128, SEG], F32, tag="ps")
                    nc.tensor.matmul(rsq_ps, ones128_bf, sqT, start=True, stop=True)
                    r_mem = work.tile([128, SEG], F32, tag="r_mem")
                    nc.vector.reciprocal_approx_fast(out=r_mem, in_=rsq_ps)

                    # rsk: per-position 1/rowsum(sk) column factors
                    rskc_ps = psSm.tile([128, 4], F32, tag="sm")
                    for c in range(4):
                        nc.tensor.matmul(
                            rskc_ps[:, c : c + 1],
                            skT[:, c * 128 : (c + 1) * 128],
                            ones128_bf[:, 0:1],
                            start=True,
                            stop=True,
                        )
                    rsk = work.tile([128, 4], F32, tag="rsk")
                    nc.vector.reciprocal_approx_fast(out=rsk, in_=rskc_ps)

                    # ---- attention numerators ----
                    dotp_ps = psDM.tile([128, SEG], F32, tag="ps")
                    for c in range(4):
                        nc.tensor.matmul(
                            dotp_ps, vc[:, c, 0:128], eT[:, c, :], start=(c == 0), stop=(c == 3)
                        )
                    memp_ps = psDM.tile([128, SEG], F32, tag="ps")
                    nc.tensor.matmul(memp_ps, mem_bf, sqT, start=True, stop=True)

                    # ---- blend into assembled att_flat^T ----
                    # (omb / bg*r1col column factors fold into the stt scalar)
                    d2 = work.tile([128, SEG], BF16, tag="d2")
                    nc.vector.scalar_tensor_tensor(
                        out=d2, in0=dotp_ps, scalar=ombc[:, h : h + 1], in1=r_dot,
                        op0=ALU.mult, op1=ALU.mult,
                    )
                    m2 = work.tile([128, SEG], BF16, tag="m2")
                    nc.vector.scalar_tensor_tensor(
                        out=m2, in0=memp_ps, scalar=bgr1[:, h : h + 1], in1=r_mem,
                        op0=ALU.mult, op1=ALU.mult,
                    )
                    nc.vector.tensor_add(
                        af[:, :, h * 32 : (h + 1) * 32],
                        m2.rearrange("d (r t) -> d r t", t=32),
                        d2.rearrange("d (r t) -> d r t", t=32),
                    )

                    # ---- delta rule ----
                    corr_ps = psA.tile([128, 4 * DV], F32, tag="ps")
                    for c in range(4):
                        nc.tensor.matmul(
                            corr_ps[:, c * DV : (c + 1) * DV],
                            skT[:, c * 128 : (c + 1) * 128],
                            mem_bf,
                            start=True,
                            stop=True,
                        )
                    for c in range(4):
                        nc.vector.tensor_scalar_mul(
                            vc[:, c, 128:256], corr_ps[:, c * DV : (c + 1) * DV],
                            rsk[:, c : c + 1],
                        )

                    sqc = work.tile([128, 4, 128], BF16, tag="sqc")
                    for c in range(4):
                        pt = psSm.tile([128, 128], BF16, tag="sm")
                        nc.tensor.transpose(pt, sqT[:, c * 128 : (c + 1) * 128], ident_bf)
                        if c % 2 == 0:
                            nc.scalar.copy(sqc[:, c, :], pt)
                        else:
                            nc.vector.tensor_copy(sqc[:, c, :], pt)

                    dAB_ps = psSm.tile([128, 2 * DV], F32, tag="sm")
                    for c in range(4):
                        nc.tensor.matmul(
                            dAB_ps, sqc[:, c, :], vc[:, c, :], start=(c == 0), stop=(c == 3)
                        )
                    dB = work.tile([128, DV], F32, tag="dB")
                    nc.vector.tensor_mul(dB, dAB_ps[:, DV : 2 * DV], rep_all[hf][:, hh, :])
                    dA = work.tile([128, DV], F32, tag="dA")
                    nc.vector.tensor_sub(dA, dAB_ps[:, 0:DV], dB)
                    nc.vector.tensor_add(mem[:, h, :], mem[:, h, :], dA)

                    # fused phase 3 for this head-half (st=hf) once its 4
                    # heads' af columns are written
                    if hh == 3:
                        st = hf
                        ost = outst.tile([128, D], F32, tag="ost")
                        for mc in range(4):
                            po = psDM.tile([128, 512], F32, tag="ps")
                            for r in range(NKK):
                                nc.tensor.matmul(
                                    po,
                                    af[:, r, st * 128 : (st + 1) * 128],
                                    wo_res[:, r, mc * 512 : (mc + 1) * 512],
                                    start=(r == 0),
                                    stop=(r == NKK - 1),
                                )
                            if mc % 2 == 0:
                                nc.scalar.copy(ost[:, mc * 512 : (mc + 1) * 512], po)
                            else:
                                nc.vector.tensor_copy(ost[:, mc * 512 : (mc + 1) * 512], po)
                        nc.sync.dma_start(
                            out=out[r0 + st * 128 : r0 + (st + 1) * 128, :], in_=ost
                        )


    nc.finalize()
    return nc


_NC_CACHE = {}


def _get_nc():
    if "nc" not in _NC_CACHE:
        _NC_CACHE["nc"] = build_core_program()
    return _NC_CACHE["nc"]


def _make_in_maps(inputs):
    x = np.ascontiguousarray(np.asarray(inputs["x"], dtype=np.float32))
    Wq = np.ascontiguousarray(
        np.asarray(inputs["Wq"], dtype=np.float32).astype(ml_dtypes.bfloat16)
    )
    Wk = np.ascontiguousarray(
        np.asarray(inputs["Wk"], dtype=np.float32).astype(ml_dtypes.bfloat16)
    )
    Wv = np.ascontiguousarray(
        np.asarray(inputs["Wv"], dtype=np.float32).astype(ml_dtypes.bfloat16)
    )
    Wo = np.ascontiguousarray(
        np.asarray(inputs["Wo"], dtype=np.float32).astype(ml_dtypes.bfloat16)
    )
    betas = np.asarray(inputs["betas"], dtype=np.float32)

    bg = (1.0 / (1.0 + np.exp(-betas.astype(np.float64)))).astype(np.float32)
    bg = bg.reshape(16, 128)  # (H, DV)
    omb = (1.0 - bg).astype(np.float32)

    ident = np.eye(128, dtype=np.float32)
    ones = np.ones((128, 128), dtype=np.float32)
    ones_bf = np.ones((128, 128), dtype=ml_dtypes.bfloat16)
    zeros = np.zeros((128, NH * DV + NH), dtype=np.float32)

    in_maps = []
    for c in range(8):
        b, hh = c // 2, c % 2
        hsl = slice(hh * NH, (hh + 1) * NH)
        # x slab: this head-half's time rows of every segment, transposed
        xp = x[b].reshape(NSEG, 2, 256, D)[:, hh].reshape(TC, D)
        xpT = np.ascontiguousarray(xp.T.astype(ml_dtypes.bfloat16))
        in_maps.append(
            {
                "xt": xpT,
                "wq": Wq,
                "wk": Wk,
                "wv": Wv,
                "wo": Wo,
                "bg_cols": np.ascontiguousarray(bg[hsl].T),
                "omb_cols": np.ascontiguousarray(omb[hsl].T),
                "ident": ident,
                "ones_in": ones,
                "ones_bf": ones_bf,
                "ident_bf": np.eye(128, dtype=ml_dtypes.bfloat16),
                "zeros_in": zeros,
            }
        )
    return in_maps


def kernel(x, Wq, Wk, Wv, Wo, betas):
    inputs = {"x": x, "Wq": Wq, "Wk": Wk, "Wv": Wv, "Wo": Wo, "betas": betas}
    in_maps = _make_in_maps(inputs)
    nc = _get_nc()
    res = run_bass_kernel_spmd(nc, in_maps, core_ids=list(range(8)))
    B = np.asarray(x).shape[0]
    out = np.empty((B, T, D), dtype=np.float32)
    for b in range(B):
        ob = out[b].reshape(NSEG, 2, 256, D)
        ob[:, 0] = res.results[2 * b]["out"].reshape(NSEG, 256, D)
        ob[:, 1] = res.results[2 * b + 1]["out"].reshape(NSEG, 256, D)
    return out


# revision 32
# speedup vs baseline: 1.0431x; 1.0431x over previous
"""CompressiveMemory (Infini-attention style) Bass kernel for 8x TRN2 NeuronCores.

Problem (hardcoded): B=4, T=4096, D=2048, H=16, DK=DV=128, SEG=512, NSEG=8.

The reference's _split_heads is a RAW reshape (B, SEG, H*dk) -> (B, H, SEG, dk):
head h's (SEG, dk) matrix is segment time rows [h*32, (h+1)*32) of the full
(SEG, 2048) projection, reinterpreted as 512 rows of 128. Attention is
permutation-equivariant over sequence positions, so we use the hardware-
friendly ordering s' = r*32 + tau (r = col-chunk 0..15, tau = time row 0..31);
q/k/v/att all use the same ordering and the result is identical.

Sharding: 8 cores = 4 batches x 2 head-halves. Core c handles batch c//2 and
heads (c%2)*8..+8, i.e. time rows [hh*256, (hh+1)*256) of every segment.
Output rows are disjoint across cores; the host just interleaves them.

v2 design notes (vs the first working version):
 - x is transposed on the host; phase 1 runs with 512-wide moving operands
   and no PE transposes (LDWEIGHTS hidden under matmuls).
 - v scratch and the whole att/Wo path are bf16 (PE same speed, half DMA);
   q/k scratch stays f32 for the softmax/elu path.
 - phase 2: denominators are computed PE-replicated (ones128 stationary, no
   extra PE cost since the moving dim is unchanged) and inverted with
   reciprocal_approx_fast on [128,512] tiles -- no single-partition
   reciprocals. (1-bg) and bg/(1+cum) column factors fold into the ScalarE
   PSUM->SBUF copies. The delta-rule matmul runs 256-wide over a
   [v | corr*rsk] concatenated rhs to dodge the fp32r narrow-moving penalty.
 - phase 3 (out = att @ Wo, bf16) is fused into the per-segment loop; att
   never round-trips to DRAM.

Math notes vs the reference:
 - z always has identical rows: z[., c] = 1 + cum[c]; only the cum vector is
   tracked. sq@z = outer(rowsum(sq), 1+cum), sk@z likewise.
 - softmax without max subtraction (scores*scale ~ N(0,1), exp safe in fp32).
 - elu(x)+1 = relu(x) + exp(-relu(-x)).
 - delta update: mem += sq^T@v - (sq^T@(corr*rsk_col))*r1_row, corr = sk@mem.
"""

import os
from contextlib import ExitStack

import ml_dtypes
import numpy as np

import concourse.bass as bass
import concourse.mybir as mybir
import concourse.tile as tile
from concourse import bacc
from concourse.bass_utils import run_bass_kernel_spmd

F32 = mybir.dt.float32
F32R = mybir.dt.float32r
BF16 = mybir.dt.bfloat16
AF = mybir.ActivationFunctionType
ALU = mybir.AluOpType

# Global problem dims
T = 4096
D = 2048
SEG = 512
NSEG = T // SEG
NH = 8            # local heads per core
DK = 128
DV = 128
TC = NSEG * 256   # 2048 core-local time rows (256 per segment)
NKK = D // 128    # 16 contraction chunks
SCALE = 1.0 / np.sqrt(128.0)


def build_core_program():
    phases = os.environ.get("BASS_PHASES", "12")
    nc = bacc.Bacc("TRN2", target_bir_lowering=False)

    xt = nc.dram_tensor("xt", [D, TC], BF16, kind="ExternalInput")
    wq = nc.dram_tensor("wq", [D, D], BF16, kind="ExternalInput")
    wk = nc.dram_tensor("wk", [D, D], BF16, kind="ExternalInput")
    wv = nc.dram_tensor("wv", [D, D], BF16, kind="ExternalInput")
    wo = nc.dram_tensor("wo", [D, D], BF16, kind="ExternalInput")
    bg_cols = nc.dram_tensor("bg_cols", [DV, NH], F32, kind="ExternalInput")
    omb_cols = nc.dram_tensor("omb_cols", [DV, NH], F32, kind="ExternalInput")
    ident_in = nc.dram_tensor("ident", [128, 128], F32, kind="ExternalInput")
    ones_in = nc.dram_tensor("ones_in", [128, 128], F32, kind="ExternalInput")
    ones_bf_in = nc.dram_tensor("ones_bf", [128, 128], BF16, kind="ExternalInput")
    ident_bf_in = nc.dram_tensor("ident_bf", [128, 128], BF16, kind="ExternalInput")
    zeros_in = nc.dram_tensor("zeros_in", [128, NH * DV + NH], F32, kind="ExternalInput")
    out = nc.dram_tensor("out", [TC, D], F32, kind="ExternalOutput")

    with ExitStack() as ctx:
        ctx.enter_context(
            nc.allow_low_precision(reason="float32r/bf16 tiles feed the PE")
        )
        tc = ctx.enter_context(tile.TileContext(nc))

        # ---- DRAM scratch ----
        dram = ctx.enter_context(tc.tile_pool(name="dram", bufs=1, space="DRAM"))
        qT_scr = dram.tile([NKK, 128, TC], BF16)  # P_q^T as (jc, dd, t)
        kT_scr = dram.tile([NKK, 128, TC], BF16)
        v_scr = dram.tile([TC, D], BF16)          # P_v natural, bf16

        # ---- constants ----
        const = ctx.enter_context(tc.tile_pool(name="const", bufs=1))
        ident = const.tile([128, 128], F32R)
        nc.sync.dma_start(out=ident, in_=ident_in[:, :].bitcast(F32R))
        ones128 = const.tile([128, 128], F32R)
        nc.sync.dma_start(out=ones128, in_=ones_in[:, :].bitcast(F32R))
        ones128_bf = const.tile([128, 128], BF16)
        nc.sync.dma_start(out=ones128_bf, in_=ones_bf_in[:, :])
        ones_row_bf = const.tile([1, 128], BF16)
        nc.sync.dma_start(out=ones_row_bf, in_=ones_bf_in[0:1, :])
        ident_bf = const.tile([128, 128], BF16)
        nc.sync.dma_start(out=ident_bf, in_=ident_bf_in[:, :])

        # =========================== PHASE 1 ===========================
        # Three passes; one full (D, D) projection weight resident per pass.
        # q/k emit transposed f32 scratch; v emits natural bf16 scratch.
        with ExitStack() as p1w:
          wpool = (
              p1w.enter_context(tc.tile_pool(name="w_p1", bufs=2))
              if "1" in phases
              else None
          )
          for which, w_dram in ((("q", wq), ("k", wk), ("v", wv)) if "1" in phases else ()):
            with ExitStack() as p1:
                sb = p1.enter_context(tc.tile_pool(name=f"sb_{which}", bufs=2))
                st_pool = p1.enter_context(tc.tile_pool(name=f"st_{which}", bufs=4))
                ps = p1.enter_context(tc.tile_pool(name=f"ps_{which}", bufs=5, space="PSUM"))

                wt = wpool.tile([128, NKK, D], BF16, tag="wt")
                w_view = w_dram.rearrange("(kk p) j -> p kk j", p=128)
                if which == "q":
                    # chunked: the first jc group starts after chunk 0 lands
                    for kk in range(NKK):
                        nc.gpsimd.dma_start(out=wt[:, kk, :], in_=w_view[:, kk, :])
                else:
                    nc.gpsimd.dma_start(out=wt, in_=w_view)

                for g in range(4):  # 512 t-cols per group
                    c0 = g * 512
                    xg = sb.tile([128, NKK, 512], BF16, tag="xg")
                    nc.sync.dma_start(
                        out=xg,
                        in_=xt[:, c0 : c0 + 512].rearrange("(kk p) t -> p kk t", p=128),
                    )
                    if which in ("q", "k"):
                        scr = qT_scr if which == "q" else kT_scr
                        for jc in range(NKK):
                            pq = ps.tile([128, 512], F32, tag="proj")
                            for kk in range(NKK):
                                nc.tensor.matmul(
                                    pq,
                                    wt[:, kk, jc * 128 : (jc + 1) * 128],
                                    xg[:, kk, :],
                                    start=(kk == 0),
                                    stop=(kk == NKK - 1),
                                )
                            qs = st_pool.tile([128, 512], BF16, tag="stage")
                            if jc % 2 == 0:
                                nc.scalar.copy(qs, pq)
                            else:
                                nc.vector.tensor_copy(qs, pq)
                            nc.sync.dma_start(out=scr[jc, :, c0 : c0 + 512], in_=qs)
                    else:
                        for tch in range(4):  # t-chunks of 128 within group
                            r0 = c0 + tch * 128
                            for mc in range(4):
                                pv = ps.tile([128, 512], F32, tag="proj")
                                for kk in range(NKK):
                                    nc.tensor.matmul(
                                        pv,
                                        xg[:, kk, tch * 128 : (tch + 1) * 128],
                                        wt[:, kk, mc * 512 : (mc + 1) * 512],
                                        start=(kk == 0),
                                        stop=(kk == NKK - 1),
                                    )
                                vs = st_pool.tile([128, 512], BF16, tag="stage")
                                if (tch + mc) % 2 == 0:
                                    nc.scalar.copy(vs, pv)
                                else:
                                    nc.vector.tensor_copy(vs, pv)
                                nc.sync.dma_start(
                                    out=v_scr[r0 : r0 + 128, mc * 512 : (mc + 1) * 512],
                                    in_=vs,
                                )

        # ================= PHASE 2+3 (fused, per segment) =================
        with ExitStack() as p2:
          if "2" in phases:
            res = p2.enter_context(tc.tile_pool(name="res", bufs=1))
            wpool3 = p2.enter_context(tc.tile_pool(name="w_o", bufs=1))
            qk = p2.enter_context(tc.tile_pool(name="qk", bufs=3))
            vcp = p2.enter_context(tc.tile_pool(name="vcp", bufs=2))
            etp = p2.enter_context(tc.tile_pool(name="etp", bufs=3))
            work = p2.enter_context(tc.tile_pool(name="work", bufs=2))
            segw = p2.enter_context(tc.tile_pool(name="segw", bufs=2))
            afp = p2.enter_context(tc.tile_pool(name="afp", bufs=2))
            outst = p2.enter_context(tc.tile_pool(name="outst", bufs=2))
            psA = p2.enter_context(tc.tile_pool(name="psA", bufs=2, space="PSUM"))
            psDen = p2.enter_context(tc.tile_pool(name="psDen", bufs=2, space="PSUM"))
            psDM = p2.enter_context(tc.tile_pool(name="psDM", bufs=2, space="PSUM"))
            psSm = p2.enter_context(tc.tile_pool(name="psSm", bufs=2, space="PSUM"))

            # residents
            wo_res = wpool3.tile([128, NKK, D], BF16, tag="wo")
            nc.sync.dma_start(out=wo_res, in_=wo.rearrange("(r p) m -> p r m", p=128))
            bgc = res.tile([128, NH], F32)
            nc.sync.dma_start(out=bgc, in_=bg_cols[:, :])
            ombc = res.tile([128, NH], F32)
            nc.sync.dma_start(out=ombc, in_=omb_cols[:, :])
            mem = res.tile([128, NH, DV], F32R)  # (dk, h, dv)
            nc.sync.dma_start(
                out=mem,
                in_=zeros_in[:, 0 : NH * DV].rearrange("p (h v) -> p h v", h=NH).bitcast(F32R),
            )
            cum = res.tile([128, NH], F32R)      # (dk, h)
            nc.sync.dma_start(out=cum, in_=zeros_in[:, NH * DV :].bitcast(F32R))

            for seg in range(NSEG):
                r0 = seg * 256
                # ---- per-segment loads (two head-halves of 4 heads) ----
                vcb = []
                for hf in range(2):
                    rows = slice(r0 + hf * 128, r0 + hf * 128 + 128)
                    # v | corrS combo tile; left 128 of each 256 slab = v
                    vc4 = vcp.tile([128, 4, 4, 256], BF16, tag="vc")
                    v_base = v_scr[rows, :].rearrange(
                        "(hh t) (c rr d) -> rr hh t c d", hh=4, c=4, rr=4, d=128
                    )
                    for rr in range(4):
                        for hh4 in range(4):
                            nc.gpsimd.dma_start(
                                out=vc4[rr * 32 : (rr + 1) * 32, hh4, :, 0:128],
                                in_=v_base[rr, hh4],
                            )
                    vcb.append(vc4)

                # ---- cum-derived factors for all heads (OLD cum) ----
                cumT_ps = psSm.tile([NH, 128], F32R, tag="sm")
                nc.tensor.transpose(cumT_ps, cum[:, :], ident)
                z1row8 = segw.tile([NH, 128], F32, tag="z1row8")
                nc.scalar.add(z1row8, cumT_ps, 1.0)
                r1row8_bf = segw.tile([NH, 128], BF16, tag="r1row8b")
                r1tmp = segw.tile([NH, 128], F32, tag="r1tmp")
                nc.vector.reciprocal_approx_fast(out=r1tmp, in_=z1row8)
                nc.vector.tensor_copy(r1row8_bf, r1tmp)
                # flatten to one partition so per-head slices sit at base 0
                r1flat = segw.tile([1, NH, 128], BF16, tag="r1flat")
                nc.sync.dma_start(out=r1flat, in_=r1row8_bf[:, :])
                z1col = segw.tile([128, NH], F32, tag="z1col")
                nc.scalar.add(z1col, cum[:, :], 1.0)
                r1col = segw.tile([128, NH], F32, tag="r1col")
                nc.vector.reciprocal_approx_fast(out=r1col, in_=z1col)
                bgr1 = segw.tile([128, NH], F32, tag="bgr1")
                nc.vector.tensor_mul(bgr1, bgc, r1col)

                # assembled att_flat^T for this segment (bf16)
                af = afp.tile([128, NKK, 256], BF16, tag="af")

                rep_all = []
                for hf in range(2):
                    rep_ps4 = psDen.tile([128, SEG], F32, tag="ps")
                    nc.tensor.matmul(
                        rep_ps4,
                        ones_row_bf,
                        r1flat[:, hf * 4 : hf * 4 + 4, :].rearrange("o h d -> o (h d)"),
                        start=True,
                        stop=True,
                    )
                    rep4 = segw.tile([128, 4, 128], F32, tag=f"rep4_{hf}")
                    nc.scalar.copy(rep4, rep_ps4)
                    rep_all.append(rep4)

                for h in range(NH):
                    hf, hh = h // 4, h % 4
                    tr0 = r0 + hf * 128 + hh * 32
                    vc = vcb[hf][:, hh]          # [128, 4, 256] bf16

                    # per-head q/k loads into one tile (s' = r*32 + tau)
                    qkT = qk.tile([128, 2, NKK, 32], BF16, tag="qkT")
                    nc.gpsimd.dma_start(
                        out=qkT[:, 0],
                        in_=qT_scr[:, :, tr0 : tr0 + 32].rearrange("r d t -> d r t"),
                    )
                    nc.gpsimd.dma_start(
                        out=qkT[:, 1],
                        in_=kT_scr[:, :, tr0 : tr0 + 32].rearrange("r d t -> d r t"),
                    )
                    qkF = qkT.rearrange("p a r t -> p (a r t)")
                    qT = qkT[:, 0].rearrange("p r t -> p (r t)")
                    kT3 = qkT[:, 1]

                    # ---- sq/sk = elu+1, one batched chain for q and k ----
                    sqsk = work.tile([128, 2, SEG], BF16, tag="sqsk")
                    nrelu = work.tile([128, 2 * SEG], F32, tag="nrelu")
                    nc.scalar.activation(out=nrelu, in_=qkF, func=AF.Relu, scale=-1.0)
                    texp = work.tile([128, 2 * SEG], F32, tag="texp")
                    nc.scalar.activation(out=texp, in_=nrelu, func=AF.Exp, scale=-1.0)
                    nc.vector.scalar_tensor_tensor(
                        out=sqsk.rearrange("p a t -> p (a t)"), in0=qkF, scalar=0.0,
                        in1=texp, op0=ALU.max, op1=ALU.add,
                    )
                    sqT = sqsk[:, 0]
                    skT = sqsk[:, 1]

                    # cum += colsum(sk); the z1/bgr1 reads above used OLD cum
                    csk = work.tile([128, 1], F32, tag="csk")
                    nc.vector.reduce_sum(csk, skT, axis=mybir.AxisListType.X)
                    nc.vector.tensor_add(cum[:, h : h + 1], cum[:, h : h + 1], csk)

                    # mem in bf16 (plus a ones column) for the corr matmul
                    mem_bf1 = work.tile([128, DV + 1], BF16, tag="mem_bf1")
                    nc.scalar.copy(mem_bf1[:, 0:DV], mem[:, h, :])
                    nc.vector.tensor_copy(mem_bf1[:, DV : DV + 1], ones128_bf[:, 0:1])
                    mem_bf = mem_bf1[:, 0:DV]

                    # ---- scores^T -> e^T (bf16) ----
                    eT = etp.tile([128, 4, SEG], BF16, tag="eT")
                    for c in range(4):
                        pscT = psA.tile([128, SEG], F32, tag="ps")
                        nc.tensor.matmul(
                            pscT,
                            kT3[:, c * 4 : c * 4 + 4, :].rearrange("p r t -> p (r t)"),
                            qT,
                            start=True,
                            stop=True,
                        )
                        nc.scalar.activation(out=eT[:, c, :], in_=pscT, func=AF.Exp, scale=SCALE)

                    # ---- replicated denominators ----
                    den_ps = psDen.tile([128, SEG], F32, tag="ps")
                    for c in range(4):
                        nc.tensor.matmul(
                            den_ps, ones128_bf, eT[:, c, :], start=(c == 0), stop=(c == 3)
                        )
                    r_dot = work.tile([128, SEG], F32, tag="r_dot")
                    nc.vector.reciprocal_approx_fast(out=r_dot, in_=den_ps)

                    rsq_ps = psDen.tile([128, SEG], F32, tag="ps")
                    nc.tensor.matmul(rsq_ps, ones128_bf, sqT, start=True, stop=True)
                    r_mem = work.tile([128, SEG], F32, tag="r_mem")
                    nc.vector.reciprocal_approx_fast(out=r_mem, in_=rsq_ps)

                    # ---- attention numerators ----
                    dotp_ps = psDM.tile([128, SEG], F32, tag="ps")
                    for c in range(4):
                        nc.tensor.matmul(
                            dotp_ps, vc[:, c, 0:128], eT[:, c, :], start=(c == 0), stop=(c == 3)
                        )
                    memp_ps = psDM.tile([128, SEG], F32, tag="ps")
                    nc.tensor.matmul(memp_ps, mem_bf, sqT, start=True, stop=True)

                    # ---- blend into assembled att_flat^T ----
                    # (omb / bg*r1col column factors fold into the stt scalar)
                    d2 = work.tile([128, SEG], BF16, tag="d2")
                    nc.vector.scalar_tensor_tensor(
                        out=d2, in0=dotp_ps, scalar=ombc[:, h : h + 1], in1=r_dot,
                        op0=ALU.mult, op1=ALU.mult,
                    )
                    m2 = work.tile([128, SEG], BF16, tag="m2")
                    nc.vector.scalar_tensor_tensor(
                        out=m2, in0=memp_ps, scalar=bgr1[:, h : h + 1], in1=r_mem,
                        op0=ALU.mult, op1=ALU.mult,
                    )
                    nc.vector.tensor_add(
                        af[:, :, h * 32 : (h + 1) * 32],
                        m2.rearrange("d (r t) -> d r t", t=32),
                        d2.rearrange("d (r t) -> d r t", t=32),
                    )

                    # ---- delta rule: corr plus rowsum(sk) in one matmul ----
                    corr01 = psA.tile([128, 2, DV + 1], F32, tag="ps")
                    corr23 = psSm.tile([128, 2, DV + 1], F32, tag="sm")
                    for c in range(4):
                        dst_ps = corr01 if c < 2 else corr23
                        nc.tensor.matmul(
                            dst_ps[:, c % 2, :],
                            skT[:, c * 128 : (c + 1) * 128],
                            mem_bf1,
                            start=True,
                            stop=True,
                        )
                    rsk = work.tile([128, 4], F32, tag="rsk")
                    nc.vector.reciprocal_approx_fast(
                        out=rsk[:, 0:2],
                        in_=corr01[:, :, DV : DV + 1].rearrange("p a o -> p (a o)"),
                    )
                    nc.vector.reciprocal_approx_fast(
                        out=rsk[:, 2:4],
                        in_=corr23[:, :, DV : DV + 1].rearrange("p a o -> p (a o)"),
                    )
                    for c in range(4):
                        dst_ps = corr01 if c < 2 else corr23
                        nc.vector.tensor_scalar_mul(
                            vc[:, c, 128:256], dst_ps[:, c % 2, 0:DV],
                            rsk[:, c : c + 1],
                        )

                    sqc = work.tile([128, 4, 128], BF16, tag="sqc")
                    for c in range(4):
                        pt = psSm.tile([128, 128], BF16, tag="sm")
                        nc.tensor.transpose(pt, sqT[:, c * 128 : (c + 1) * 128], ident_bf)
                        if c % 2 == 0:
                            nc.scalar.copy(sqc[:, c, :], pt)
                        else:
                            nc.vector.tensor_copy(sqc[:, c, :], pt)

                    dAB_ps = psSm.tile([128, 2 * DV], F32, tag="sm")
                    for c in range(4):
                        nc.tensor.matmul(
                            dAB_ps, sqc[:, c, :], vc[:, c, :], start=(c == 0), stop=(c == 3)
                        )
                    dB = work.tile([128, DV], F32, tag="dB")
                    nc.vector.tensor_mul(dB, dAB_ps[:, DV : 2 * DV], rep_all[hf][:, hh, :])
                    dA = work.tile([128, DV], F32, tag="dA")
                    nc.vector.tensor_sub(dA, dAB_ps[:, 0:DV], dB)
                    nc.vector.tensor_add(mem[:, h, :], mem[:, h, :], dA)

                # ---- fused phase 3: out rows for this segment ----
                for st in range(2):
                    ost = outst.tile([128, D], F32, tag="ost")
                    for mc in range(4):
                        po = psDM.tile([128, 512], F32, tag="ps")
                        for r in range(NKK):
                            nc.tensor.matmul(
                                po,
                                af[:, r, st * 128 : (st + 1) * 128],
                                wo_res[:, r, mc * 512 : (mc + 1) * 512],
                                start=(r == 0),
                                stop=(r == NKK - 1),
                            )
                        if mc % 2 == 0:
                            nc.scalar.copy(ost[:, mc * 512 : (mc + 1) * 512], po)
                        else:
                            nc.vector.tensor_copy(ost[:, mc * 512 : (mc + 1) * 512], po)
                    nc.sync.dma_start(
                        out=out[r0 + st * 128 : r0 + (st + 1) * 128, :], in_=ost
                    )


    nc.finalize()
    return nc


_NC_CACHE = {}


def _get_nc():
    if "nc" not in _NC_CACHE:
        _NC_CACHE["nc"] = build_core_program()
    return _NC_CACHE["nc"]


def _make_in_maps(inputs):
    x = np.ascontiguousarray(np.asarray(inputs["x"], dtype=np.float32))
    Wq = np.ascontiguousarray(
        np.asarray(inputs["Wq"], dtype=np.float32).astype(ml_dtypes.bfloat16)
    )
    Wk = np.ascontiguousarray(
        np.asarray(inputs["Wk"], dtype=np.float32).astype(ml_dtypes.bfloat16)
    )
    Wv = np.ascontiguousarray(
        np.asarray(inputs["Wv"], dtype=np.float32).astype(ml_dtypes.bfloat16)
    )
    Wo = np.ascontiguousarray(
        np.asarray(inputs["Wo"], dtype=np.float32).astype(ml_dtypes.bfloat16)
    )
    betas = np.asarray(inputs["betas"], dtype=np.float32)

    bg = (1.0 / (1.0 + np.exp(-betas.astype(np.float64)))).astype(np.float32)
    bg = bg.reshape(16, 128)  # (H, DV)
    omb = (1.0 - bg).astype(np.float32)

    ident = np.eye(128, dtype=np.float32)
    ones = np.ones((128, 128), dtype=np.float32)
    ones_bf = np.ones((128, 128), dtype=ml_dtypes.bfloat16)
    zeros = np.zeros((128, NH * DV + NH), dtype=np.float32)

    in_maps = []
    for c in range(8):
        b, hh = c // 2, c % 2
        hsl = slice(hh * NH, (hh + 1) * NH)
        # x slab: this head-half's time rows of every segment, transposed
        xp = x[b].reshape(NSEG, 2, 256, D)[:, hh].reshape(TC, D)
        xpT = np.ascontiguousarray(xp.T.astype(ml_dtypes.bfloat16))
        in_maps.append(
            {
                "xt": xpT,
                "wq": Wq,
                "wk": Wk,
                "wv": Wv,
                "wo": Wo,
                "bg_cols": np.ascontiguousarray(bg[hsl].T),
                "omb_cols": np.ascontiguousarray(omb[hsl].T),
                "ident": ident,
                "ones_in": ones,
                "ones_bf": ones_bf,
                "ident_bf": np.eye(128, dtype=ml_dtypes.bfloat16),
                "zeros_in": zeros,
            }
        )
    return in_maps


def kernel(x, Wq, Wk, Wv, Wo, betas):
    inputs = {"x": x, "Wq": Wq, "Wk": Wk, "Wv": Wv, "Wo": Wo, "betas": betas}
    in_maps = _make_in_maps(inputs)
    nc = _get_nc()
    res = run_bass_kernel_spmd(nc, in_maps, core_ids=list(range(8)))
    B = np.asarray(x).shape[0]
    out = np.empty((B, T, D), dtype=np.float32)
    for b in range(B):
        ob = out[b].reshape(NSEG, 2, 256, D)
        ob[:, 0] = res.results[2 * b]["out"].reshape(NSEG, 256, D)
        ob[:, 1] = res.results[2 * b + 1]["out"].reshape(NSEG, 256, D)
    return out
